# revision 15
# baseline (speedup 1.0000x reference)
"""Trainium2 Bass kernel for ConditionalFeedForward (MoE routed FFN).

Problem: M=2048 tokens, D=1024, I=2048, E=8 experts, TOPK=2.
out[t, s] = FFN_{e}(x[t]) with e = expert_indices[t, s], where
FFN_e(x) = (silu(x @ w1_e.T) * (x @ w3_e.T)) @ w2_e.T  (w13 = [w1; w3]).

Strategy: expert-quarter parallelism over 8 cores.
 - Host routes tokens to experts (dedup across the two slots), then splits
   every expert into 4 QUARTERS along the intermediate dim I.  Experts are
   sorted by token count and paired (1st with 2nd, 3rd with 4th, ...);
   slot s of core c processes quarter (c%4) of expert pair[s][c//4].  All
   cores run the same program with per-slot capacities C_0..C_3 = the
   larger count of each pair - near-perfect load balance (the old whole-
   expert mapping padded every core to the hottest expert's count).
   Each quarter emits a PARTIAL output over the full model dim; the host
   sums the 4 quarters per expert and scatters to the (token, slot) grid.
 - Everything on the PE path is bf16 (1 col/cycle stream; fp8 DoubleRow
   measured 5.9% rel err end-to-end vs the 2e-2 gate - unusable).  PSUM
   accumulates fp32; silu*gate on ACT+DVE in fp32, g requantized to bf16.
 - Startup: slot 0's x and w13 ride in 8 per-k PIECES (piece k = one
   contiguous DRAM slab [x_k | wA,wB slices of its 4 pairs]), so the
   first matmul waits only on piece 0; piece 0 is further split into two
   DMAs (x+pairs01 / pairs23) to land the first 4 matmuls even earlier.
   The PE consumes 8 matmuls per piece - faster than arrival - zero
   stalls.  All input DMAs ride the sync HWDGE ring in need-order.
 - PE warmup: the HAM clock gate holds the PE at 1.2 GHz until it's been
   busy ~3.4us; scratch matmuls run from the tile-context entry (~7.2us)
   until piece 0 lands (~10us).
 - Tail: the last slot's last d-block runs in two column chunks whose
   copy+DMA chains use disjoint engines (DVE+sync / ACT+scalar ring).

Measured on the staged inputs: ~93-94 us HW exec (baseline 114.9 us,
prior bests 98.7/98.3 us), rel l2 err ~3.7e-3 vs fp64 (gate 2e-2).
Breakdown: ~77us PE stream at the bf16 roofline (384 matmuls x
(497+488+472+466)/4 cols) + ~1us cold-clock + ~2.8us head + ~3.4us tail
(DMA completion latency) + ~9.2us fixed framework preamble/teardown
(paid even by an empty kernel - measured 14.5us for a 2-DMA program).
"""

import os

import numpy as np
import ml_dtypes

import concourse.bass as bass
import concourse.tile as tile
from concourse import bacc, mybir
from concourse.bass_utils import run_bass_kernel_spmd

M, D, I, E, TOPK = 2048, 1024, 2048, 8, 2
P = 128
KD = D // P            # 8   k-tiles over D (mm1 contraction)
NI2 = (2 * I) // P     # 32  n-tiles over 2I (mm1 output rows)
NPAIR = NI2 // 2       # 16  (x1, x3) pairs
KI = I // P            # 16  k-tiles over I (mm2 contraction)
ND = D // P            # 8   d-tiles over D (mm2 output rows)

NSLOT = 4              # expert-quarters per core
QPAIR = NPAIR // NSLOT  # 4 (x1,x3) pairs per quarter
QKI = KI // NSLOT       # 4 mm2 k-tiles per quarter

WARMUP_MMS = 11        # scratch 256-col matmuls before real data lands

F32 = mybir.dt.float32
BF16 = mybir.dt.bfloat16
NP_BF16 = ml_dtypes.bfloat16

# exec time of the most recent launch (ns), populated when BASS_TRACE=1
LAST_EXEC_TIME_NS = None

_program_cache = {}


def _build_program(Cs):
    """Cs: tuple of 4 per-slot token capacities (each <= 512)."""
    assert len(Cs) == NSLOT and all(64 <= c <= 512 for c in Cs)
    C0 = Cs[0]
    PW = C0 + 2 * QPAIR * P   # slot-0 piece width: x_k + 4 pairs' k-slices
    H0 = C0 + 2 * P           # split point of piece 0 (x + pair-0 slices)
    nc = bacc.Bacc(
        "TRN2",
        target_bir_lowering=False,
        debug=False,
        enable_asserts=False,
        num_devices=E,
    )

    # slot-0 pieces: xk[k] = [ x_k | wA0_k | wB0_k | ... | wA3_k | wB3_k ]
    xk_d = nc.dram_tensor("xk", (KD, P, PW), BF16, kind="ExternalInput").ap()
    # slots 1-3: x packed host-side in SBUF image order [P, KD*C_s]
    xs_d = [
        nc.dram_tensor(f"xs{s}", (P, KD * Cs[s]), BF16, kind="ExternalInput").ap()
        for s in range(1, NSLOT)
    ]
    # slots 1-3 w13: one merged slab per slot (fewer DMAs -> fewer
    # DMA-lane semaphore re-arms on the sync queue, which otherwise
    # couple late input DMA issue to slow out-DMA completions)
    w13_d = [
        nc.dram_tensor(
            f"w13q{s}", (P, QPAIR * 2 * KD * P), BF16, kind="ExternalInput"
        ).ap()
        for s in range(1, NSLOT)
    ]
    # per-slot w2 quarter, merged: [P, (ND//2) * 2*QKI*P]
    w2_d = [
        nc.dram_tensor(
            f"w2q{s}", (P, (ND // 2) * 2 * QKI * P), BF16, kind="ExternalInput"
        ).ap()
        for s in range(NSLOT)
    ]
    # partial outputs in bf16: 4 quarter-partials per expert are summed
    # on the host in fp32; the extra rounding is ~0.2% rel - far inside
    # the 2e-2 gate - and it halves the out-DMA traffic, which otherwise
    # contends with the input stream and stalls slot-2/3 weight delivery.
    # layout [P, ND*C]: partition-major so one SBUF tile image maps to
    # one contiguous DMA; host reshapes/transposes when reducing.
    out_d = [
        nc.dram_tensor(f"outT{s}", (P, ND * Cs[s]), BF16, kind="ExternalOutput").ap()
        for s in range(NSLOT)
    ]

    # warmup operand lives outside the tile pools: its memset runs on the
    # Pool queue in the framework preamble (~6.3us), before the tile
    # context's entry barrier, so the first warmup matmul issues at tile
    # entry (~7.3us) with no intra-context dependency.
    zt_h = nc.alloc_sbuf_tensor("ztw", [P, 256], BF16)
    nc.gpsimd.memset(zt_h.ap(), 0.0)
    zt = zt_h.ap()

    with tile.TileContext(nc) as tc:
        with (
            tc.tile_pool(name="xg", bufs=1) as xg_pool,
            tc.tile_pool(name="wt", bufs=1) as w_pool,
            tc.tile_pool(name="tmp", bufs=4) as tmp_pool,
            tc.tile_pool(name="ps", bufs=8, space="PSUM") as ps_pool,
        ):
            # ---- PE warmup: scratch matmuls on the preamble-memset tile ----
            for _ in range(WARMUP_MMS):
                psw = ps_pool.tile([P, 256], F32, tag="ps", name="ps")
                nc.tensor.matmul(
                    psw, zt[:, :P], zt[:], start=True, stop=True
                )

            # ---- tiles ----
            pieces = [
                xg_pool.tile([P, PW], BF16, tag=f"pc{k}", name=f"pc{k}")
                for k in range(KD)
            ]
            xs_t = [
                xg_pool.tile([P, KD * Cs[s]], BF16, tag=f"xs{s}", name=f"xs{s}")
                for s in range(1, NSLOT)
            ]
            w13s = {
                s: w_pool.tile(
                    [P, QPAIR * 2 * KD * P], BF16, tag=f"w13_{s}", name=f"w13_{s}"
                )
                for s in range(1, NSLOT)
            }
            w2s = {
                s: w_pool.tile(
                    [P, (ND // 2) * 2 * QKI * P], BF16, tag=f"w2_{s}", name=f"w2_{s}"
                )
                for s in range(NSLOT)
            }
            # dedicated per-slot output staging: no buffer recycling, so a
            # psO->bf16 cast never waits on an earlier chunk's out-DMA.
            ot_s = [
                xg_pool.tile(
                    [P, ND * Cs[s]], BF16, tag=f"ot{s}", name=f"ot{s}"
                )
                for s in range(NSLOT)
            ]
            g_tiles = {
                (s, pr): xg_pool.tile(
                    [P, Cs[s]], BF16, tag=f"g{s}_{pr}", name=f"g{s}_{pr}"
                )
                for s in range(NSLOT)
                for pr in range(QPAIR)
            }

            # ---- all input DMAs on the sync HWDGE ring, need-order ----
            # piece 0 split in two so the very first matmuls only wait on
            # x_0 + pairs 0/1 (~260 KB at the cold DMA rate).
            nc.sync.dma_start(pieces[0][:, :H0], xk_d[0][:, :H0])
            nc.sync.dma_start(pieces[0][:, H0:], xk_d[0][:, H0:])
            for k in range(1, KD):
                nc.sync.dma_start(pieces[k][:], xk_d[k])
            nc.sync.dma_start(w2s[0][:], w2_d[0])
            for s in range(1, NSLOT):
                nc.sync.dma_start(xs_t[s - 1][:], xs_d[s - 1])
                nc.sync.dma_start(w13s[s][:], w13_d[s - 1])
                nc.sync.dma_start(w2s[s][:], w2_d[s])

            def x_t(s, k):
                if s == 0:
                    return pieces[k][:, :C0]
                return xs_t[s - 1][:, k * Cs[s] : (k + 1) * Cs[s]]

            def w01(k, pr, half):
                off = C0 + (2 * pr + half) * P
                return pieces[k][:, off : off + P]

            def silu_mul(s, pr, psA, psB):
                st = tmp_pool.tile([P, 512], F32, tag="s", name="s")[:, : Cs[s]]
                nc.scalar.activation(st, psA, mybir.ActivationFunctionType.Silu)
                nc.vector.tensor_mul(
                    out=g_tiles[(s, pr)][:], in0=st, in1=psB
                )

            def mm2(s, final):
                C = Cs[s]
                for d in range(ND):
                    wD = w2s[s][
                        :,
                        (d // 2) * 2 * QKI * P + (d % 2) * QKI * P :
                        (d // 2) * 2 * QKI * P + (d % 2 + 1) * QKI * P,
                    ]
                    if final and d == ND - 1 and C >= 256:
                        out_chunks = [(0, C - 64), (C - 64, 64)]
                    else:
                        out_chunks = [(0, C)]
                    for ci, (c0, cn) in enumerate(out_chunks):
                        psO = ps_pool.tile([P, 512], F32, tag="ps", name="ps")[
                            :, :cn
                        ]
                        for ki in range(QKI):
                            nc.tensor.matmul(
                                psO,
                                wD[:, ki * P : (ki + 1) * P],
                                g_tiles[(s, ki)][:, c0 : c0 + cn],
                                start=(ki == 0),
                                stop=(ki == QKI - 1),
                            )
                        ot = ot_s[s][:, d * C + c0 : d * C + c0 + cn]
                        # final d-block: the two chunks' copy+DMA chains run
                        # on disjoint engine pairs so the last 64-col chunk
                        # never queues behind the big chunk's work.
                        if final and d == ND - 1 and ci > 0:
                            nc.scalar.copy(ot, psO)
                            nc.scalar.dma_start(
                                out_d[s][:, d * C + c0 : d * C + c0 + cn], ot
                            )
                        else:
                            nc.vector.tensor_copy(ot, psO)
                            if final:
                                eng = nc.scalar if d == ND - 1 else nc.sync
                                eng.dma_start(
                                    out_d[s][:, d * C + c0 : d * C + c0 + cn], ot
                                )
                if not final:
                    # one merged out-DMA for the whole slot, on the scalar
                    # ring so the sync ring stays input-only.
                    nc.scalar.dma_start(out_d[s], ot_s[s][:])

            # ---- slot 0: pairs interleaved per-k (8 PSUM accumulation
            # groups) so each arriving piece feeds 8 matmuls back-to-back.
            ps01 = [
                ps_pool.tile([P, 512], F32, tag="ps", name="ps")[:, :C0]
                for _ in range(2 * QPAIR)
            ]
            for k in range(KD):
                for j in range(2 * QPAIR):
                    nc.tensor.matmul(
                        ps01[j],
                        w01(k, j // 2, j % 2),
                        x_t(0, k),
                        start=(k == 0),
                        stop=(k == KD - 1),
                    )
            for pr in range(QPAIR):
                silu_mul(0, pr, ps01[2 * pr], ps01[2 * pr + 1])
            mm2(0, final=False)

            # ---- slots 1-3: pair-sequential from resident slabs ----
            for s in range(1, NSLOT):
                for pr in range(QPAIR):
                    slab = w13s[s][
                        :, pr * 2 * KD * P : (pr + 1) * 2 * KD * P
                    ]
                    psA = ps_pool.tile([P, 512], F32, tag="ps", name="ps")[
                        :, : Cs[s]
                    ]
                    psB = ps_pool.tile([P, 512], F32, tag="ps", name="ps")[
                        :, : Cs[s]
                    ]
                    for half, ps_ in ((0, psA), (1, psB)):
                        for k in range(KD):
                            nc.tensor.matmul(
                                ps_,
                                slab[
                                    :,
                                    half * KD * P + k * P : half * KD * P
                                    + (k + 1) * P,
                                ],
                                x_t(s, k),
                                start=(k == 0),
                                stop=(k == KD - 1),
                            )
                    silu_mul(s, pr, psA, psB)
                mm2(s, final=(s == NSLOT - 1))

    nc.compile()
    return nc


def _get_program(Cs):
    if Cs not in _program_cache:
        _program_cache[Cs] = _build_program(Cs)
    return _program_cache[Cs]


def _ensure_ntff_hook():
    """Provide antenv.axon_hooks if the image lacks it, so trace=True works."""
    import sys
    import types

    try:
        import antenv.axon_hooks  # noqa: F401

        return
    except ImportError:
        pass
    try:
        import antenv
        from trn_agent_boot.trn_boot import _ntff_profile_via_ctypes

        mod = types.ModuleType("antenv.axon_hooks")
        state = {"hook": None}
        mod.set_axon_ntff_profile_hook = lambda h: state.__setitem__("hook", h)
        mod.get_axon_ntff_profile_hook = lambda: state["hook"]
        sys.modules["antenv.axon_hooks"] = mod
        antenv.axon_hooks = mod
        mod.set_axon_ntff_profile_hook(
            _ntff_profile_via_ctypes("/opt/axon/libaxon_pjrt.so")
        )
    except Exception:
        pass


def kernel(x, w13, w2, expert_indices):
    global LAST_EXEC_TIME_NS
    x = np.asarray(x, dtype=np.float32)
    w13 = np.asarray(w13, dtype=np.float32)
    w2 = np.asarray(w2, dtype=np.float32)
    idx = np.asarray(expert_indices)
    idx32 = idx.astype(np.int64)

    m, d_model = x.shape
    e, two_i, _ = w13.shape
    inter = w2.shape[2]
    topk = idx.shape[1]
    assert (m, d_model, e, two_i, inter, topk) == (M, D, E, 2 * I, I, TOPK)

    # ---- host routing: unique (token, expert) work items per expert ----
    tok_unique = [
        np.unique(np.concatenate([np.nonzero(idx32[:, s] == ei)[0] for s in range(topk)]))
        for ei in range(E)
    ]
    cnts = np.array([len(u) for u in tok_unique])
    order = np.argsort(-cnts, kind="stable")          # experts, hottest first
    # slot s processes experts order[2s] (cores 0-3) and order[2s+1]
    # (cores 4-7); capacity = the hotter of the pair.
    Cs = tuple(max(64, int(cnts[order[2 * s]])) for s in range(NSLOT))

    nc = _get_program(Cs)

    # pre-transposed per-expert activations/weights (built once per expert)
    xT_e, w13p_e, w2t_e = {}, {}, {}
    for ei in range(E):
        tok_ids = tok_unique[ei]
        cnt = len(tok_ids)
        slot = int(np.nonzero(order == ei)[0][0]) // 2
        C = Cs[slot]
        xg = np.zeros((C, D), dtype=np.float32)
        xg[:cnt] = x[tok_ids]
        xT_e[ei] = np.ascontiguousarray(
            xg.T.reshape(KD, P, C).transpose(1, 0, 2).astype(NP_BF16)
        )                                            # [p, k, c]

        A4 = w13[ei].astype(NP_BF16).reshape(NI2, P, KD, P)   # [n, c, k, p]
        w13t = A4.transpose(0, 3, 2, 1).reshape(NI2, P, KD * P)
        w13p_e[ei] = np.ascontiguousarray(
            np.concatenate([w13t[:NPAIR], w13t[NPAIR:]], axis=2)
        )                                            # [pair, p, 2*KD*P]

        B4 = w2[ei].astype(NP_BF16).reshape(ND, P, KI, P)     # [d, c, ki, p]
        w2t_e[ei] = B4.transpose(0, 3, 2, 1)                  # [d, p, ki, p]

    in_maps = []
    for c in range(E):
        q = c % 4                      # quarter index this core handles
        imap = {}
        for s in range(NSLOT):
            ei = int(order[2 * s + c // 4])
            C = Cs[s]
            xT = xT_e[ei]
            w13p = w13p_e[ei]                         # [16, P, 2*KD*P]
            prs = range(q * QPAIR, (q + 1) * QPAIR)   # this quarter's pairs
            kis = range(q * QKI, (q + 1) * QKI)       # this quarter's mm2 k

            if s == 0:
                # pieces: [ x_k | wA,wB slices of the quarter's 4 pairs ]
                xk = np.empty((KD, P, C + 2 * QPAIR * P), dtype=NP_BF16)
                for k in range(KD):
                    xk[k, :, :C] = xT[:, k]
                    for j, pr in enumerate(prs):
                        for half in range(2):
                            src = w13p[pr][
                                :, half * KD * P + k * P : half * KD * P + (k + 1) * P
                            ]
                            col = C + (2 * j + half) * P
                            xk[k, :, col : col + P] = src
                imap["xk"] = xk
            else:
                imap[f"xs{s}"] = np.ascontiguousarray(
                    xT.reshape(P, KD * C)
                )
                imap[f"w13q{s}"] = np.ascontiguousarray(
                    np.concatenate([w13p[pr] for pr in prs], axis=1)
                )

            # w2 quarter: [ND//2, P, 2*QKI*P], d-pair fused, ki sliced
            w2q = w2t_e[ei][:, :, list(kis)]          # [d, p, QKI, p]
            w2q = w2q.reshape(ND, P, QKI * P)
            w2q = w2q.reshape(ND // 2, 2, P, QKI * P).transpose(0, 2, 1, 3).reshape(
                ND // 2, P, 2 * QKI * P
            )
            imap[f"w2q{s}"] = np.ascontiguousarray(
                w2q.transpose(1, 0, 2).reshape(P, -1)
            )
        in_maps.append(imap)

    trace = bool(os.environ.get("BASS_TRACE"))
    if trace:
        _ensure_ntff_hook()
    res = run_bass_kernel_spmd(nc, in_maps, core_ids=list(range(E)), trace=trace)
    LAST_EXEC_TIME_NS = res.exec_time_ns

    # ---- host reduce + scatter ----
    # expert order[2s + g] partials live on cores g*4 .. g*4+3 (slot s).
    out = np.empty((M, topk, D), dtype=np.float32)
    for s in range(NSLOT):
        for g in range(2):
            ei = int(order[2 * s + g])
            cnt = len(tok_unique[ei])
            acc = np.zeros((D, Cs[s]), dtype=np.float32)
            for qq in range(4):
                c = g * 4 + qq
                arr = res.results[c][f"outT{s}"].reshape(P, ND, Cs[s])
                acc += arr.transpose(1, 0, 2).reshape(D, Cs[s]).astype(np.float32)
            oe = acc[:, :cnt].T.astype(np.float32)   # [cnt, D]
            for sl in range(topk):
                sel = np.nonzero(idx32[:, sl] == ei)[0]
                out[sel, sl] = oe[np.searchsorted(tok_unique[ei], sel)]

    return out


# revision 16
# speedup vs baseline: 1.2089x; 1.2089x over previous
"""Trainium2 Bass kernel for ConditionalFeedForward (MoE routed FFN).

Problem: M=2048 tokens, D=1024, I=2048, E=8 experts, TOPK=2.
out[t, s] = FFN_{e}(x[t]) with e = expert_indices[t, s], where
FFN_e(x) = (silu(x @ w1_e.T) * (x @ w3_e.T)) @ w2_e.T  (w13 = [w1; w3]).

Strategy: expert-quarter parallelism over 8 cores.
 - Host routes tokens to experts (dedup across the two slots), then splits
   every expert into 4 QUARTERS along the intermediate dim I.  Experts are
   sorted by token count and paired (1st with 2nd, 3rd with 4th, ...);
   slot s of core c processes quarter (c%4) of expert pair[s][c//4].  All
   cores run the same program with per-slot capacities C_0..C_3 = the
   larger count of each pair - near-perfect load balance (the old whole-
   expert mapping padded every core to the hottest expert's count).
   Each quarter emits a PARTIAL output over the full model dim; the host
   sums the 4 quarters per expert and scatters to the (token, slot) grid.
 - Everything on the PE path is bf16 (1 col/cycle stream; fp8 DoubleRow
   measured 5.9% rel err end-to-end vs the 2e-2 gate - unusable).  PSUM
   accumulates fp32; silu*gate on ACT+DVE in fp32, g requantized to bf16.
 - Startup: slot 0's x and w13 ride in 8 per-k PIECES (piece k = one
   contiguous DRAM slab [x_k | wA,wB slices of its 4 pairs]), so the
   first matmul waits only on piece 0; piece 0 is further split into two
   DMAs (x+pairs01 / pairs23) to land the first 4 matmuls even earlier.
   The PE consumes 8 matmuls per piece - faster than arrival - zero
   stalls.  All input DMAs ride the sync HWDGE ring in need-order.
 - PE warmup: the HAM clock gate holds the PE at 1.2 GHz until it's been
   busy ~3.4us; scratch matmuls run from the tile-context entry (~7.2us)
   until piece 0 lands (~10us).
 - Tail: the last slot's last d-block runs in two column chunks whose
   copy+DMA chains use disjoint engines (DVE+sync / ACT+scalar ring).

Measured on the staged inputs: ~93-94 us HW exec (baseline 114.9 us,
prior bests 98.7/98.3 us), rel l2 err ~3.7e-3 vs fp64 (gate 2e-2).
Breakdown: ~77us PE stream at the bf16 roofline (384 matmuls x
(497+488+472+466)/4 cols) + ~1us cold-clock + ~2.8us head + ~3.4us tail
(DMA completion latency) + ~9.2us fixed framework preamble/teardown
(paid even by an empty kernel - measured 14.5us for a 2-DMA program).
"""

import os

import numpy as np
import ml_dtypes

import concourse.bass as bass
import concourse.tile as tile
from concourse import bacc, mybir
from concourse.bass_utils import run_bass_kernel_spmd

M, D, I, E, TOPK = 2048, 1024, 2048, 8, 2
P = 128
KD = D // P            # 8   k-tiles over D (mm1 contraction)
NI2 = (2 * I) // P     # 32  n-tiles over 2I (mm1 output rows)
NPAIR = NI2 // 2       # 16  (x1, x3) pairs
KI = I // P            # 16  k-tiles over I (mm2 contraction)
ND = D // P            # 8   d-tiles over D (mm2 output rows)

NSLOT = 4              # expert-quarters per core
QPAIR = NPAIR // NSLOT  # 4 (x1,x3) pairs per quarter
QKI = KI // NSLOT       # 4 mm2 k-tiles per quarter

WARMUP_MMS = 12        # scratch 256-col matmuls before real data lands

F32 = mybir.dt.float32
BF16 = mybir.dt.bfloat16
NP_BF16 = ml_dtypes.bfloat16

# exec time of the most recent launch (ns), populated when BASS_TRACE=1
LAST_EXEC_TIME_NS = None

_program_cache = {}


def _build_program(Cs):
    """Cs: tuple of 4 per-slot token capacities (each <= 512)."""
    assert len(Cs) == NSLOT and all(64 <= c <= 512 for c in Cs)
    C0 = Cs[0]
    PW = C0 + 2 * QPAIR * P   # slot-0 piece width: x_k + 4 pairs' k-slices
    H0 = C0 + QPAIR * P       # split point of piece 0 (x + pairs 0-1)
    nc = bacc.Bacc(
        "TRN2",
        target_bir_lowering=False,
        debug=False,
        enable_asserts=False,
        num_devices=E,
    )

    # slot-0 pieces: xk[k] = [ x_k | wA0_k | wB0_k | ... | wA3_k | wB3_k ]
    xk_d = nc.dram_tensor("xk", (KD, P, PW), BF16, kind="ExternalInput").ap()
    # slots 1-3: x packed host-side in SBUF image order [P, KD*C_s]
    xs_d = [
        nc.dram_tensor(f"xs{s}", (P, KD * Cs[s]), BF16, kind="ExternalInput").ap()
        for s in range(1, NSLOT)
    ]
    # slots 1-3 w13: one merged slab per slot (fewer DMAs -> fewer
    # DMA-lane semaphore re-arms on the sync queue, which otherwise
    # couple late input DMA issue to slow out-DMA completions)
    w13_d = [
        nc.dram_tensor(
            f"w13q{s}", (P, QPAIR * 2 * KD * P), BF16, kind="ExternalInput"
        ).ap()
        for s in range(1, NSLOT)
    ]
    # per-slot w2 quarter, merged: [P, (ND//2) * 2*QKI*P]
    w2_d = [
        nc.dram_tensor(
            f"w2q{s}", (P, (ND // 2) * 2 * QKI * P), BF16, kind="ExternalInput"
        ).ap()
        for s in range(NSLOT)
    ]
    # partial outputs in bf16: 4 quarter-partials per expert are summed
    # on the host in fp32; the extra rounding is ~0.2% rel - far inside
    # the 2e-2 gate - and it halves the out-DMA traffic, which otherwise
    # contends with the input stream and stalls slot-2/3 weight delivery.
    # layout [P, ND*C]: partition-major so one SBUF tile image maps to
    # one contiguous DMA; host reshapes/transposes when reducing.
    out_d = [
        nc.dram_tensor(f"outT{s}", (P, ND * Cs[s]), BF16, kind="ExternalOutput").ap()
        for s in range(NSLOT)
    ]

    with tile.TileContext(nc) as tc:
        with (
            tc.tile_pool(name="xg", bufs=1) as xg_pool,
            tc.tile_pool(name="wt", bufs=1) as w_pool,
            tc.tile_pool(name="tmp", bufs=4) as tmp_pool,
            tc.tile_pool(name="ps", bufs=8, space="PSUM") as ps_pool,
        ):
            # ---- PE warmup: scratch matmuls on a memset tile ----
            zt = xg_pool.tile([P, 256], BF16, tag="zt", name="zt")
            nc.gpsimd.memset(zt[:], 0.0)
            for _ in range(WARMUP_MMS):
                psw = ps_pool.tile([P, 256], F32, tag="ps", name="ps")
                nc.tensor.matmul(
                    psw, zt[:, :P], zt[:], start=True, stop=True
                )

            # ---- tiles ----
            pieces = [
                xg_pool.tile([P, PW], BF16, tag=f"pc{k}", name=f"pc{k}")
                for k in range(KD)
            ]
            xs_t = [
                xg_pool.tile([P, KD * Cs[s]], BF16, tag=f"xs{s}", name=f"xs{s}")
                for s in range(1, NSLOT)
            ]
            w13s = {
                s: w_pool.tile(
                    [P, QPAIR * 2 * KD * P], BF16, tag=f"w13_{s}", name=f"w13_{s}"
                )
                for s in range(1, NSLOT)
            }
            w2s = {
                s: w_pool.tile(
                    [P, (ND // 2) * 2 * QKI * P], BF16, tag=f"w2_{s}", name=f"w2_{s}"
                )
                for s in range(NSLOT)
            }
            # dedicated per-slot output staging: no buffer recycling, so a
            # psO->bf16 cast never waits on an earlier chunk's out-DMA.
            ot_s = [
                xg_pool.tile(
                    [P, ND * Cs[s]], BF16, tag=f"ot{s}", name=f"ot{s}"
                )
                for s in range(NSLOT)
            ]
            g_tiles = {
                (s, pr): xg_pool.tile(
                    [P, Cs[s]], BF16, tag=f"g{s}_{pr}", name=f"g{s}_{pr}"
                )
                for s in range(NSLOT)
                for pr in range(QPAIR)
            }

            # ---- all input DMAs on the sync HWDGE ring, need-order ----
            # piece 0 split in two so the very first matmuls only wait on
            # x_0 + pairs 0/1 (~260 KB at the cold DMA rate).
            nc.sync.dma_start(pieces[0][:, :H0], xk_d[0][:, :H0])
            nc.sync.dma_start(pieces[0][:, H0:], xk_d[0][:, H0:])
            for k in range(1, KD):
                nc.sync.dma_start(pieces[k][:], xk_d[k])
            nc.sync.dma_start(w2s[0][:], w2_d[0])
            for s in range(1, NSLOT):
                nc.sync.dma_start(xs_t[s - 1][:], xs_d[s - 1])
                nc.sync.dma_start(w13s[s][:], w13_d[s - 1])
                nc.sync.dma_start(w2s[s][:], w2_d[s])

            def x_t(s, k):
                if s == 0:
                    return pieces[k][:, :C0]
                return xs_t[s - 1][:, k * Cs[s] : (k + 1) * Cs[s]]

            def w01(k, pr, half):
                off = C0 + (2 * pr + half) * P
                return pieces[k][:, off : off + P]

            def silu_mul(s, pr, psA, psB):
                st = tmp_pool.tile([P, 512], F32, tag="s", name="s")[:, : Cs[s]]
                nc.scalar.activation(st, psA, mybir.ActivationFunctionType.Silu)
                nc.vector.tensor_mul(
                    out=g_tiles[(s, pr)][:], in0=st, in1=psB
                )

            def mm2(s, final):
                C = Cs[s]
                for d in range(ND):
                    wD = w2s[s][
                        :,
                        (d // 2) * 2 * QKI * P + (d % 2) * QKI * P :
                        (d // 2) * 2 * QKI * P + (d % 2 + 1) * QKI * P,
                    ]
                    if final and d == ND - 1 and C >= 256:
                        out_chunks = [(0, C - 64), (C - 64, 64)]
                    else:
                        out_chunks = [(0, C)]
                    for ci, (c0, cn) in enumerate(out_chunks):
                        psO = ps_pool.tile([P, 512], F32, tag="ps", name="ps")[
                            :, :cn
                        ]
                        for ki in range(QKI):
                            nc.tensor.matmul(
                                psO,
                                wD[:, ki * P : (ki + 1) * P],
                                g_tiles[(s, ki)][:, c0 : c0 + cn],
                                start=(ki == 0),
                                stop=(ki == QKI - 1),
                            )
                        ot = ot_s[s][:, d * C + c0 : d * C + c0 + cn]
                        # final d-block: the two chunks' copy+DMA chains run
                        # on disjoint engine pairs so the last 64-col chunk
                        # never queues behind the big chunk's work.
                        if final and d == ND - 1 and ci > 0:
                            nc.scalar.copy(ot, psO)
                            nc.scalar.dma_start(
                                out_d[s][:, d * C + c0 : d * C + c0 + cn], ot
                            )
                        else:
                            nc.vector.tensor_copy(ot, psO)
                            if final:
                                eng = nc.scalar if d == ND - 1 else nc.sync
                                eng.dma_start(
                                    out_d[s][:, d * C + c0 : d * C + c0 + cn], ot
                                )
                if not final:
                    # one merged out-DMA for the whole slot, on the scalar
                    # ring so the sync ring stays input-only.
                    nc.scalar.dma_start(out_d[s], ot_s[s][:])

            # ---- slot 0: pairs interleaved per-k (8 PSUM accumulation
            # groups) so each arriving piece feeds 8 matmuls back-to-back.
            ps01 = [
                ps_pool.tile([P, 512], F32, tag="ps", name="ps")[:, :C0]
                for _ in range(2 * QPAIR)
            ]
            for k in range(KD):
                for j in range(2 * QPAIR):
                    nc.tensor.matmul(
                        ps01[j],
                        w01(k, j // 2, j % 2),
                        x_t(0, k),
                        start=(k == 0),
                        stop=(k == KD - 1),
                    )
            for pr in range(QPAIR):
                silu_mul(0, pr, ps01[2 * pr], ps01[2 * pr + 1])
            mm2(0, final=False)

            # ---- slots 1-3: pair-sequential from resident slabs ----
            for s in range(1, NSLOT):
                for pr in range(QPAIR):
                    slab = w13s[s][
                        :, pr * 2 * KD * P : (pr + 1) * 2 * KD * P
                    ]
                    psA = ps_pool.tile([P, 512], F32, tag="ps", name="ps")[
                        :, : Cs[s]
                    ]
                    psB = ps_pool.tile([P, 512], F32, tag="ps", name="ps")[
                        :, : Cs[s]
                    ]
                    for half, ps_ in ((0, psA), (1, psB)):
                        for k in range(KD):
                            nc.tensor.matmul(
                                ps_,
                                slab[
                                    :,
                                    half * KD * P + k * P : half * KD * P
                                    + (k + 1) * P,
                                ],
                                x_t(s, k),
                                start=(k == 0),
                                stop=(k == KD - 1),
                            )
                    silu_mul(s, pr, psA, psB)
                mm2(s, final=(s == NSLOT - 1))

    nc.compile()
    return nc


def _get_program(Cs):
    if Cs not in _program_cache:
        _program_cache[Cs] = _build_program(Cs)
    return _program_cache[Cs]


def _ensure_ntff_hook():
    """Provide antenv.axon_hooks if the image lacks it, so trace=True works."""
    import sys
    import types

    try:
        import antenv.axon_hooks  # noqa: F401

        return
    except ImportError:
        pass
    try:
        import antenv
        from trn_agent_boot.trn_boot import _ntff_profile_via_ctypes

        mod = types.ModuleType("antenv.axon_hooks")
        state = {"hook": None}
        mod.set_axon_ntff_profile_hook = lambda h: state.__setitem__("hook", h)
        mod.get_axon_ntff_profile_hook = lambda: state["hook"]
        sys.modules["antenv.axon_hooks"] = mod
        antenv.axon_hooks = mod
        mod.set_axon_ntff_profile_hook(
            _ntff_profile_via_ctypes("/opt/axon/libaxon_pjrt.so")
        )
    except Exception:
        pass


def kernel(x, w13, w2, expert_indices):
    global LAST_EXEC_TIME_NS
    x = np.asarray(x, dtype=np.float32)
    w13 = np.asarray(w13, dtype=np.float32)
    w2 = np.asarray(w2, dtype=np.float32)
    idx = np.asarray(expert_indices)
    idx32 = idx.astype(np.int64)

    m, d_model = x.shape
    e, two_i, _ = w13.shape
    inter = w2.shape[2]
    topk = idx.shape[1]
    assert (m, d_model, e, two_i, inter, topk) == (M, D, E, 2 * I, I, TOPK)

    # ---- host routing: unique (token, expert) work items per expert ----
    tok_unique = [
        np.unique(np.concatenate([np.nonzero(idx32[:, s] == ei)[0] for s in range(topk)]))
        for ei in range(E)
    ]
    cnts = np.array([len(u) for u in tok_unique])
    order = np.argsort(-cnts, kind="stable")          # experts, hottest first
    # slot s processes experts order[2s] (cores 0-3) and order[2s+1]
    # (cores 4-7); capacity = the hotter of the pair.
    Cs = tuple(max(64, int(cnts[order[2 * s]])) for s in range(NSLOT))

    nc = _get_program(Cs)

    # pre-transposed per-expert activations/weights (built once per expert)
    xT_e, w13p_e, w2t_e = {}, {}, {}
    for ei in range(E):
        tok_ids = tok_unique[ei]
        cnt = len(tok_ids)
        slot = int(np.nonzero(order == ei)[0][0]) // 2
        C = Cs[slot]
        xg = np.zeros((C, D), dtype=np.float32)
        xg[:cnt] = x[tok_ids]
        xT_e[ei] = np.ascontiguousarray(
            xg.T.reshape(KD, P, C).transpose(1, 0, 2).astype(NP_BF16)
        )                                            # [p, k, c]

        A4 = w13[ei].astype(NP_BF16).reshape(NI2, P, KD, P)   # [n, c, k, p]
        w13t = A4.transpose(0, 3, 2, 1).reshape(NI2, P, KD * P)
        w13p_e[ei] = np.ascontiguousarray(
            np.concatenate([w13t[:NPAIR], w13t[NPAIR:]], axis=2)
        )                                            # [pair, p, 2*KD*P]

        B4 = w2[ei].astype(NP_BF16).reshape(ND, P, KI, P)     # [d, c, ki, p]
        w2t_e[ei] = B4.transpose(0, 3, 2, 1)                  # [d, p, ki, p]

    in_maps = []
    for c in range(E):
        q = c % 4                      # quarter index this core handles
        imap = {}
        for s in range(NSLOT):
            ei = int(order[2 * s + c // 4])
            C = Cs[s]
            xT = xT_e[ei]
            w13p = w13p_e[ei]                         # [16, P, 2*KD*P]
            prs = range(q * QPAIR, (q + 1) * QPAIR)   # this quarter's pairs
            kis = range(q * QKI, (q + 1) * QKI)       # this quarter's mm2 k

            if s == 0:
                # pieces: [ x_k | wA,wB slices of the quarter's 4 pairs ]
                xk = np.empty((KD, P, C + 2 * QPAIR * P), dtype=NP_BF16)
                for k in range(KD):
                    xk[k, :, :C] = xT[:, k]
                    for j, pr in enumerate(prs):
                        for half in range(2):
                            src = w13p[pr][
                                :, half * KD * P + k * P : half * KD * P + (k + 1) * P
                            ]
                            col = C + (2 * j + half) * P
                            xk[k, :, col : col + P] = src
                imap["xk"] = xk
            else:
                imap[f"xs{s}"] = np.ascontiguousarray(
                    xT.reshape(P, KD * C)
                )
                imap[f"w13q{s}"] = np.ascontiguousarray(
                    np.concatenate([w13p[pr] for pr in prs], axis=1)
                )

            # w2 quarter: [ND//2, P, 2*QKI*P], d-pair fused, ki sliced
            w2q = w2t_e[ei][:, :, list(kis)]          # [d, p, QKI, p]
            w2q = w2q.reshape(ND, P, QKI * P)
            w2q = w2q.reshape(ND // 2, 2, P, QKI * P).transpose(0, 2, 1, 3).reshape(
                ND // 2, P, 2 * QKI * P
            )
            imap[f"w2q{s}"] = np.ascontiguousarray(
                w2q.transpose(1, 0, 2).reshape(P, -1)
            )
        in_maps.append(imap)

    trace = bool(os.environ.get("BASS_TRACE"))
    if trace:
        _ensure_ntff_hook()
    res = run_bass_kernel_spmd(nc, in_maps, core_ids=list(range(E)), trace=trace)
    LAST_EXEC_TIME_NS = res.exec_time_ns

    # ---- host reduce + scatter ----
    # expert order[2s + g] partials live on cores g*4 .. g*4+3 (slot s).
    out = np.empty((M, topk, D), dtype=np.float32)
    for s in range(NSLOT):
        for g in range(2):
            ei = int(order[2 * s + g])
            cnt = len(tok_unique[ei])
            acc = np.zeros((D, Cs[s]), dtype=np.float32)
            for qq in range(4):
                c = g * 4 + qq
                arr = res.results[c][f"outT{s}"].reshape(P, ND, Cs[s])
                acc += arr.transpose(1, 0, 2).reshape(D, Cs[s]).astype(np.float32)
            oe = acc[:, :cnt].T.astype(np.float32)   # [cnt, D]
            for sl in range(topk):
                sel = np.nonzero(idx32[:, sl] == ei)[0]
                out[sel, sl] = oe[np.searchsorted(tok_unique[ei], sel)]

    return out
